# revision 5
# baseline (speedup 1.0000x reference)
"""Trainium2 Bass kernel for ragged KeyQueryAttention pooling (v2).

Math (per batch b):
    logits[t] = ||x_t @ U1||^2 - ||x_t @ U2||^2,  U1 = (K+Q)/2, U2 = (K-Q)/2
    att = softmax(logits over valid t);  out[b] = sum_t att[t] * x[t, :] + bias

Device strategy (8 NeuronCores, data-parallel over batch):
  - B=64 batches sorted by length (desc), dealt round-robin into 8 slots
    per core; slot chunk count n4_j = ceil(group_max_len/128) rounded up
    to a multiple of 4. One SPMD program, value-specialized on n4_list.
  - Slot DMA: one contiguous transfer; partition p holds DRAM rows
    [p*n4, (p+1)*n4). Chunk c = {row p*n4 + c}_p (permutation along T is
    irrelevant to the math; masks follow the same mapping).
  - float32r everywhere on the PE: 1 cycle/row at >=256 moving columns
    (vs 4 for fp32), full fp32 bit layout with 11-bit mantissa.
  - Per 4-chunk group: 4 PE transposes (x chunk stationary, f32r identity
    moving) -> xT [128,512] PSUM; ScalarE copy -> SBUF f32r; one proj
    matmul kq^T (stationary) x xT (moving 512) -> G^T [2L,512] PSUM;
    ScalarE Square -> Gsq f32r SBUF; one sign-matmul (one-hot +/-1
    stationary band) accumulating logits^T rows into a per-slot [G,512]
    PSUM stack.
  - Per slot: one DVE copy of the stack, 4 tiny PE transposes -> logits
    columns [128, n4]; DVE mask-add; DVE rowmax; GpSimd partition
    all-reduce(max); DVE negate; ScalarE exp (bias=-max, accum_out=zrow).
  - Weighted sum: per group one f32r matmul att4 (stationary [128,4]) x
    X4 (moving [128,512]) accumulated into a per-slot [4,512] PSUM tile;
    diagonal blocks hold sum_t att*x for chunks c = g (mod 4). Host sums
    the 4 diagonal blocks, divides by z = sum(zrow), adds bias.
  - PE instruction stream is slot-pipelined: slot j's softmax/weighted
    tail is emitted in the middle of slot j+1's logits phase so the PE
    never stalls on the softmax latency chain.
"""

import os
import numpy as np

import concourse.bass as bass
import concourse.bacc as bacc
import concourse.tile as tile
from concourse import mybir
from concourse import bass_isa
from concourse.bass_utils import run_bass_kernel_spmd

B, T, D, L = 64, 8192, 128, 64
NCORES = 8
SLOTS = B // NCORES  # 8 slots per core
F32 = mybir.dt.float32
F32R = mybir.dt.float32r
AF = mybir.ActivationFunctionType
ALU = mybir.AluOpType

LAST_EXEC_NS = None  # filled when KQA_TRACE=1

_PROG_CACHE = {}


def _build_program(n4_list):
    nc = bacc.Bacc()
    ntot = sum(n4_list)
    xs = [
        nc.declare_dram_parameter(f"x{j}", [128, n * D], F32R, isOutput=False)
        for j, n in enumerate(n4_list)
    ]
    kq = nc.declare_dram_parameter("kq", [D, 2 * L], F32R, isOutput=False)
    idr = nc.declare_dram_parameter("idr", [128, 128], F32R, isOutput=False)
    id16 = nc.declare_dram_parameter("id16", [16, 16], F32, isOutput=False)
    sgnb = nc.declare_dram_parameter("sgnb", [128, 31], F32R, isOutput=False)
    maskp = nc.declare_dram_parameter("mask", [128, ntot], F32, isOutput=False)
    outw = nc.declare_dram_parameter("outw", [4, SLOTS * 512], F32, isOutput=True)
    outz = nc.declare_dram_parameter("outz", [128, SLOTS], F32, isOutput=True)

    with tile.TileContext(nc) as tc:
        with (
            tc.tile_pool(name="consts", bufs=1) as consts,
            tc.tile_pool(name="xpool", bufs=3) as xpool,
            tc.tile_pool(name="spool", bufs=3) as spool,
            tc.tile_pool(name="tpool", bufs=2) as tpool,
            tc.tile_pool(name="psT", bufs=2, space="PSUM") as psT,
            tc.tile_pool(name="psG", bufs=2, space="PSUM") as psG,
            tc.tile_pool(name="psL", bufs=2, space="PSUM") as psL,
            tc.tile_pool(name="psC", bufs=1, space="PSUM") as psC,
            tc.tile_pool(name="psW", bufs=1, space="PSUM") as psW,
        ):
            kq_sb = consts.tile([D, 2 * L], F32R)
            nc.sync.dma_start(out=kq_sb, in_=kq[:, :])
            identr = consts.tile([128, 128], F32R)
            nc.sync.dma_start(out=identr, in_=idr[:, :])
            ident16 = consts.tile([16, 16], F32)
            nc.sync.dma_start(out=ident16, in_=id16[:, :])
            sgn_sb = consts.tile([128, 31], F32R)
            nc.sync.dma_start(out=sgn_sb, in_=sgnb[:, :])
            mask_sb = consts.tile([128, ntot], F32)
            nc.sync.dma_start(out=mask_sb, in_=maskp[:, :])
            wacc_sb = consts.tile([4, SLOTS * 512], F32)
            z_sb = consts.tile([128, SLOTS], F32)

            def make_tail(j, n4, off, x_sb, lg_sb):
                G = n4 // 4

                def tail():
                    # logits rows -> columns: 4 tiny transposes
                    lcol_ps = psC.tile([128, G, 4], F32, tag="lcol")
                    for c4 in range(4):
                        nc.tensor.transpose(
                            lcol_ps[:, :, c4],
                            lg_sb[:, c4 * 128 : (c4 + 1) * 128],
                            ident16[0:G, 0:G],
                        )
                    lcol = spool.tile([128, n4], F32, tag="lcol_sb")
                    nc.vector.tensor_tensor(
                        lcol, lcol_ps.rearrange("p a b -> p (a b)"),
                        mask_sb[:, off : off + n4],
                        op=ALU.add,
                    )
                    rmax = spool.tile([128, 1], F32, tag="rmax")
                    nc.vector.tensor_reduce(
                        rmax, lcol, axis=mybir.AxisListType.X, op=ALU.max
                    )
                    amax = spool.tile([128, 1], F32, tag="amax")
                    nc.gpsimd.partition_all_reduce(
                        amax, rmax, channels=128,
                        reduce_op=bass_isa.ReduceOp.max,
                    )
                    negm = spool.tile([128, 1], F32, tag="negm")
                    nc.vector.tensor_scalar_mul(negm, amax, -1.0)
                    p_sb = spool.tile([128, n4], F32R, tag="p")
                    nc.scalar.activation(
                        p_sb, lcol, AF.Exp, bias=negm, scale=1.0,
                        accum_out=z_sb[:, j : j + 1],
                    )
                    # weighted sum: cross-product accumulation, diag wanted
                    wacc_ps = psW.tile([4, 512], F32, tag="wacc")
                    for g in range(G):
                        nc.tensor.matmul(
                            wacc_ps,
                            p_sb[:, 4 * g : 4 * g + 4],
                            x_sb[:, 4 * g : 4 * g + 4, :],
                            start=(g == 0),
                            stop=(g == G - 1),
                            skip_group_check=True,
                        )
                    nc.vector.tensor_copy(
                        wacc_sb[:, j * 512 : (j + 1) * 512], wacc_ps
                    )

                return tail

            offs = np.concatenate([[0], np.cumsum(n4_list)]).astype(int)
            emit_order = sorted(range(SLOTS), key=lambda j: n4_list[j])

            pending = None
            for j in emit_order:
                n4 = n4_list[j]
                off = int(offs[j])
                G = n4 // 4
                x_sb = xpool.tile([128, n4, D], F32R, tag="x")
                nc.sync.dma_start(out=x_sb, in_=xs[j][:, :])
                lg_ps = psL.tile([G, 512], F32, tag="lg")
                fire_at = min(2, G - 1)

                def psq(g, xT_sb):
                    gt_ps = psG.tile([128, 512], F32, tag="gt")
                    nc.tensor.matmul(
                        gt_ps, kq_sb, xT_sb, start=True, stop=True
                    )
                    gsq_sb = tpool.tile([128, 512], F32R, tag="gsq")
                    nc.scalar.activation(gsq_sb, gt_ps, AF.Square)
                    nc.tensor.matmul(
                        lg_ps,
                        sgn_sb[:, 15 - g : 15 - g + G],
                        gsq_sb,
                        start=(g == 0),
                        stop=(g == G - 1),
                        skip_group_check=True,
                    )

                prev = None  # (g, xT_sb) awaiting proj/square/sign
                for g in range(G):
                    xT_ps = psT.tile([128, 512], F32R, tag="xT")
                    for c4 in range(4):
                        nc.tensor.transpose(
                            xT_ps[:, c4 * 128 : (c4 + 1) * 128],
                            x_sb[:, 4 * g + c4, :],
                            identr,
                        )
                    xT_sb = tpool.tile([128, 512], F32R, tag="xTs")
                    if g % 4 == 0:
                        nc.scalar.activation(xT_sb, xT_ps.bitcast(F32), AF.Copy)
                    else:
                        nc.vector.tensor_copy(xT_sb, xT_ps)
                    if prev is not None:
                        psq(*prev)
                    prev = (g, xT_sb)
                    if g == fire_at and pending is not None:
                        pending()
                        pending = None
                psq(*prev)
                lg_sb = spool.tile([G, 512], F32, tag="lgs")
                nc.vector.tensor_copy(lg_sb, lg_ps)
                pending = make_tail(j, n4, off, x_sb, lg_sb)
            pending()
            nc.sync.dma_start(out=outw[:, :], in_=wacc_sb)
            nc.sync.dma_start(out=outz[:, :], in_=z_sb)
    nc.finalize()
    return nc


def kernel(seq, lengths, key_w, query_w, bias):
    global LAST_EXEC_NS
    seq = np.ascontiguousarray(np.asarray(seq, dtype=np.float32))
    lengths_np = np.asarray(lengths).astype(np.int64)
    key_w = np.asarray(key_w, dtype=np.float32)
    query_w = np.asarray(query_w, dtype=np.float32)
    bias = np.asarray(bias, dtype=np.float32)

    order = np.argsort(-lengths_np, kind="stable")  # descending length
    n4_list = []
    for j in range(SLOTS):
        grp = order[j * NCORES : (j + 1) * NCORES]
        n = max(1, int(-(-int(lengths_np[grp].max()) // 128)))
        n4_list.append(-(-n // 4) * 4)  # round up to multiple of 4
    key = tuple(n4_list)
    if key not in _PROG_CACHE:
        _PROG_CACHE[key] = _build_program(n4_list)
    nc = _PROG_CACHE[key]

    def rnd(a):  # fp32r host rounding for small constants
        return (a.astype(np.float32).view(np.uint32) & np.uint32(0xFFFFF000)).view(
            np.float32
        )

    u1 = (key_w + query_w) * 0.5
    u2 = (key_w - query_w) * 0.5
    kqcat = rnd(np.concatenate([u1, u2], axis=1))  # [D, 2L]
    sgn = np.concatenate([np.ones(L), -np.ones(L)]).astype(np.float32)
    sgnb = np.zeros((128, 31), dtype=np.float32)
    sgnb[:, 15] = sgn
    idr = np.eye(128, dtype=np.float32)
    id16 = np.eye(16, dtype=np.float32)

    in_maps = []
    for i in range(NCORES):
        m = {"kq": kqcat, "idr": idr, "id16": id16, "sgnb": sgnb}
        mask_cols = []
        for j, n4 in enumerate(n4_list):
            b = int(order[j * NCORES + i])
            m[f"x{j}"] = seq[b, : n4 * 128, :].reshape(128, n4 * D)
            lb = int(lengths_np[b])
            # row at (partition p, chunk c) is p*n4 + c
            rows = np.arange(128)[:, None] * n4 + np.arange(n4)[None, :]
            col = np.where(rows < lb, 0.0, -1e30).astype(np.float32)
            mask_cols.append(col)
        m["mask"] = np.ascontiguousarray(np.concatenate(mask_cols, axis=1))
        in_maps.append(m)

    trace = os.environ.get("KQA_TRACE") == "1"
    res = run_bass_kernel_spmd(
        nc, in_maps, core_ids=list(range(NCORES)), trace=trace
    )
    LAST_EXEC_NS = res.exec_time_ns

    out = np.empty((B, D), dtype=np.float32)
    for i in range(NCORES):
        w = res.results[i]["outw"].reshape(4, SLOTS, 512)
        zr = res.results[i]["outz"]  # [128, SLOTS]
        for j in range(SLOTS):
            b = int(order[j * NCORES + i])
            acc = np.zeros(D, dtype=np.float64)
            for c4 in range(4):
                acc += w[c4, j, c4 * 128 : (c4 + 1) * 128].astype(np.float64)
            z = zr[:, j].astype(np.float64).sum()
            out[b] = (acc / z + bias.astype(np.float64)).astype(np.float32)
    return out
